# revision 6
# baseline (speedup 1.0000x reference)
"""BP-MLL loss kernel for Trainium2 (Bass/Tile), data-parallel over 8 NeuronCores.

Reference computation (per row r of [B, L] inputs):
    s_pos[r] = sum_{j: t=1} exp(-x[r,j])
    s_neg[r] = sum_{j: t=0} exp( x[r,j])
    n_pos[r] = #{j: t=1},  n_neg[r] = L - n_pos[r]
    loss     = sum_r s_pos[r]*s_neg[r] / (n_pos[r]*n_neg[r])

Sharding: batch dim B=8192 split 8 ways (1024 rows/core). Each core streams its
81.92 MB x/t slice once (HBM-bound, ~421 GB/s/core when unconstrained) and
emits per-(partition, chunk) partial sums; the tiny per-row combine (exact
n_pos recovery, product, divide, global sum) runs on host in float64.

Flat layout: each core's [1024, 10000] slice is viewed as a flat [10.24M]
stream and cut into [128, w] tiles whose DMA source is one fully contiguous
span (partition stride = w) - maximal DRAM page locality, uniform tiles, no
row-group bookkeeping on device. Tile widths all divide 10000 so each
partition's span stays inside one matrix row; the host scatter-adds the
per-slot sums into per-row totals.

Per-tile device work (mask folded into exp args):
    DVE:  u = C*t - x  (C = 8192 = 2^13)      accum -> su = C*n_pos - sum(x)
    ACT:  exp(u - C) = exp(-x) if t=1 else 0  accum -> s_pos
    ACT:  exp(-u)    = exp(x)  if t=0 else 0  accum -> s_neg
C is a power of 2 so C*t is exact; |sum(x)| + rounding noise in su is O(300)
<< C/2, so round(su/C) on host recovers n_pos exactly. fl(C - x) costs x half
an ulp of C (2^-11): ~1e-5 zero-mean relative noise in s_pos. exp(-C..)
flushes cleanly to 0.

Chunk schedule: 23 tiles of [128, 2500] then a long drain of 18 tiles of
[128, 1250]. Per-chunk scalar-engine slack is ~0.73*w - 730 ns: 1250-wide
chunks still drain ACT backlog while their own stt+2*exp pipeline latency is
only ~4.3 us, so the post-stream tail is minimal; narrower chunks have
negative slack (fixed per-instr costs) and stack up instead. Accumulator
tiles are per-engine (a tile written by two engines serializes them). The
three accumulators leave in three DMAs at the end; there is no on-device
epilogue (no reduce/recip/matmul tail).
"""

import numpy as np

import concourse.bacc as bacc
import concourse.tile as tile
from concourse import mybir
from concourse.bass_utils import run_bass_kernel_spmd

F32 = mybir.dt.float32
I32 = mybir.dt.int32
AF = mybir.ActivationFunctionType
ALU = mybir.AluOpType

B, L = 8192, 10000
N_CORES = 8
ROWS = B // N_CORES  # rows per core
P = 128
BIG = 8192.0  # mask scale: power of 2; exp(-8192) flushes to 0,
# and n_pos = round(su/BIG) is exact since |sum(x)| << BIG/2

# (name-suffix, tile width, n_tiles) per core
STAGES = [("m", 2500, 23), ("a", 1250, 18)]
assert sum(w * P * n for _, w, n in STAGES) == ROWS * L
for _s, _w, _n in STAGES:
    assert L % _w == 0  # partition spans must not cross row boundaries
N_SLOTS = sum(n for _, _, n in STAGES)


def build_bass(io_bufs=6, u_bufs=4, dma_only=False, e_dtype=F32, stages=STAGES):
    """Build the per-core Bass program. Same program runs SPMD on all cores."""
    n_sl = sum(n for _, _, n in stages)
    nc = bacc.Bacc("TRN2", target_bir_lowering=False, debug=False)
    xs, ts = {}, {}
    for s, w, n in stages:
        xs[s] = nc.dram_tensor(f"x{s}", [P * n, w], F32, kind="ExternalInput").ap()
        ts[s] = nc.dram_tensor(f"t{s}", [P * n, w], I32, kind="ExternalInput").ap()
    out = nc.dram_tensor("out", [P, 3 * n_sl], F32, kind="ExternalOutput").ap()

    with tile.TileContext(nc) as tc:
        with (
            tc.tile_pool(name="io", bufs=io_bufs) as io_pool,
            tc.tile_pool(name="upool", bufs=u_bufs) as u_pool,
            tc.tile_pool(name="epool", bufs=2) as e_pool,
            tc.tile_pool(name="acc", bufs=1) as acc_pool,
        ):
            # One accumulator tile per (engine, kind): a tile written by two
            # different engines serializes them (coarse cross-engine dep
            # tracking), which would stall the whole stream.
            acc_spos = acc_pool.tile([P, n_sl], F32, tag="acc_spos")
            acc_sneg = acc_pool.tile([P, n_sl], F32, tag="acc_sneg")
            acc_su = acc_pool.tile([P, n_sl], F32, tag="acc_su")
            if not dma_only:
                neg_big = acc_pool.tile([P, 1], F32, tag="neg_big")
                nc.vector.memset(neg_big[:], -BIG)

            sl = 0
            for s, w, n in stages:
                for k in range(n):
                    r0 = k * P
                    xt = io_pool.tile([P, w], F32, tag="x")
                    tt = io_pool.tile([P, w], I32, tag="t")
                    nc.sync.dma_start(xt[:], xs[s][r0 : r0 + P, :])
                    nc.sync.dma_start(tt[:], ts[s][r0 : r0 + P, :])
                    if dma_only:
                        sl += 1
                        continue
                    ut = u_pool.tile([P, w], F32, tag="u")
                    # u = C*t - x ; accum -> su
                    nc.vector.scalar_tensor_tensor(
                        ut[:],
                        tt[:],
                        BIG,
                        xt[:],
                        op0=ALU.mult,
                        op1=ALU.subtract,
                        accum_out=acc_su[:, sl : sl + 1],
                    )
                    ea = e_pool.tile([P, w], e_dtype, tag="escr")
                    # exp(u - C): t=1 -> exp(-x); t=0 -> 0
                    nc.scalar.activation(
                        ea[:],
                        ut[:],
                        AF.Exp,
                        bias=neg_big[:],
                        scale=1.0,
                        accum_out=acc_spos[:, sl : sl + 1],
                    )
                    eb = e_pool.tile([P, w], e_dtype, tag="escr")
                    # exp(-u): t=0 -> exp(x); t=1 -> 0
                    nc.scalar.activation(
                        eb[:],
                        ut[:],
                        AF.Exp,
                        scale=-1.0,
                        accum_out=acc_sneg[:, sl : sl + 1],
                    )
                    sl += 1

            if dma_only:
                for a in (acc_spos, acc_sneg, acc_su):
                    nc.vector.memset(a[:, 0:1], 0.0)
            # su completes at the last stt (before the last exps), so its DMA
            # overlaps ACT's tail; spos/sneg DMAs follow their final read-acc.
            nc.sync.dma_start(out[:, 2 * n_sl : 3 * n_sl], acc_su[:])
            nc.sync.dma_start(out[:, 0:n_sl], acc_spos[:])
            nc.sync.dma_start(out[:, n_sl : 2 * n_sl], acc_sneg[:])

    nc.compile()
    return nc


_NC_CACHE = {}


def _get_nc():
    if "nc" not in _NC_CACHE:
        _NC_CACHE["nc"] = build_bass()
    return _NC_CACHE["nc"]


def _shard_inputs(x, t, stages=STAGES):
    """x, t: [B, L] -> per-core dict of flat-layout views (zero-copy)."""
    in_maps = []
    for i in range(N_CORES):
        xi = np.ascontiguousarray(x[i * ROWS : (i + 1) * ROWS]).reshape(-1)
        ti = np.ascontiguousarray(t[i * ROWS : (i + 1) * ROWS]).reshape(-1)
        m = {}
        off = 0
        for s, w, n in stages:
            cnt = P * n * w
            m[f"x{s}"] = xi[off : off + cnt].reshape(P * n, w)
            m[f"t{s}"] = ti[off : off + cnt].reshape(P * n, w)
            off += cnt
        in_maps.append(m)
    return in_maps


def _row_idx(stages=STAGES):
    """Matrix-row index of each (partition, slot) partial sum: [P, n_slots]."""
    offs = []
    off = 0
    for s, w, n in stages:
        for _ in range(n):
            offs.append((off, w))
            off += P * w
    ri = np.empty((P, len(offs)), dtype=np.int64)
    for sl, (o, w) in enumerate(offs):
        starts = o + np.arange(P, dtype=np.int64) * w
        assert np.all(starts // L == (starts + w - 1) // L), (sl, w)
        ri[:, sl] = starts // L
    return ri


def _combine(core_outs, stages=STAGES):
    """Host epilogue: core_outs is a list of [P, 3*n_slots] f32 arrays."""
    n_sl = sum(n for _, _, n in stages)
    flat_idx = _row_idx(stages).ravel()
    total = 0.0
    for o in core_outs:
        o = np.asarray(o, dtype=np.float64)
        sums = []
        for k in range(3):
            acc = o[:, k * n_sl : (k + 1) * n_sl]
            rs = np.zeros(ROWS, dtype=np.float64)
            np.add.at(rs, flat_idx, acc.ravel())
            sums.append(rs)
        s_pos, s_neg, su = sums
        n_pos = np.rint(su / BIG)  # exact: |sum(x) + eps| << BIG/2
        n_neg = float(L) - n_pos
        total += float((s_pos * s_neg / (n_pos * n_neg)).sum())
    return np.float32(total)


def kernel(input, target):
    x = np.ascontiguousarray(np.asarray(input, dtype=np.float32))
    t = np.ascontiguousarray(np.asarray(target, dtype=np.int32))
    assert x.shape == (B, L) and t.shape == (B, L)

    nc = _get_nc()
    res = run_bass_kernel_spmd(
        nc, _shard_inputs(x, t), core_ids=list(range(N_CORES))
    )
    return _combine([res.results[i]["out"] for i in range(N_CORES)])
